# revision 1
# baseline (speedup 1.0000x reference)
# DiffusionPropagate Trainium2 Bass kernel.
#
# Math: new_pred[i,a] = 1 - prod_b(1 - P[b,a]*pred[i,b]), seeds clamped to 1,
# iterated NITER times.  Since P <= 0.01, log(1-x) = -(x + x^2/2 + ...) with
# x = P*pred truncates accurately after 2 terms.  In the complement domain
# q = 1 - pred this becomes
#   q_new = exp(q @ (P+P^2) - q^2 @ (P^2/2)) * exp(-colsum(P+P^2/2)) * (1-seed)
#         = exp(W) * D
# so one iteration is 2 matmul passes + exp + multiply.  D is host-precomputed.
#
# Distribution (8 cores): shard the output-node dim a (tensor parallel).
# Each core ships its [4096, 512] slice of P as fp8 (host->device bytes are
# the wall-clock bottleneck through the axon tunnel), derives the bf16 series
# matrices on-chip once, keeps them SBUF-resident, and computes q[:, shard].
# The [8,512] shard result is AllGather'd (batch-major layout -> fat DMA
# lines), then block-transposed on-chip with the DVE 32x32 stream transpose
# into the b-on-partitions lhsT layout the PE needs.  The DVE transpose only
# permutes within 32-partition groups, so the host pre-permutes the rows of
# A1 to match (see _b_index) -- that permutation is free.
import numpy as np
import ml_dtypes

import concourse.mybir as mybir
import concourse.tile as tile
from concourse import bacc

NCORES = 8
B = 8
N = 4096
NITER = 4
SHARD = N // NCORES          # 512
NCHUNK = N // 128            # 32 virtual contraction chunks
NT = N // 2048               # 2 sparse tiles (4 rank-blocks of 512 each)
NGRP = 16                    # A-matrix DMA/compute split (2 chunks each)
COLTILE = True               # 4 concurrent PE column-group matmul streams

BF16 = ml_dtypes.bfloat16
FP8 = ml_dtypes.float8_e4m3
A_SCALE = 1024.0  # P*1024 keeps fp8e4m3 entries in the normal range


def _b_index():
    """b_index[p, v]: global input-node index b held at partition p of virtual
    contraction chunk v, matching the layout the on-chip DVE block transpose
    produces.  v = 16*t + 4*c + J;  p = 32*r' + u;
    b = 2048*t + 512*r' + 128*c + 32*J + u."""
    p = np.arange(128)[:, None]
    v = np.arange(NCHUNK)[None, :]
    t, c, J = v >> 4, (v >> 2) & 3, v & 3
    rp, u = p >> 5, p & 31
    return 2048 * t + 512 * rp + 128 * c + 32 * J + u


def build_bass():
    nc = bacc.Bacc(num_devices=NCORES)
    bf = mybir.dt.bfloat16
    f32 = mybir.dt.float32

    f8 = mybir.dt.float8e4
    A_in = nc.dram_tensor("A1", [128, NCHUNK, SHARD], f8, kind="ExternalInput")
    q_in = nc.dram_tensor("q0", [NCORES * B, SHARD], bf, kind="ExternalInput")
    D_in = nc.dram_tensor("D", [B, SHARD], f32, kind="ExternalInput")
    if COLTILE:
        sel_in = nc.dram_tensor("sel", [128, B], f32, kind="ExternalInput")
    out = nc.dram_tensor("out", [B, SHARD], f32, kind="ExternalOutput")

    gsz = NCHUNK // NGRP
    with tile.TileContext(nc) as tc:
        with (
            tc.tile_pool(name="weights", bufs=1) as wpool,
            tc.tile_pool(name="work", bufs=2) as work,
            tc.tile_pool(name="psum", bufs=2, space="PSUM") as psum_pool,
            tc.tile_pool(name="dram", bufs=NITER - 1, space="DRAM") as dram,
        ):
            def load_q(src_ap):
                """src_ap: [64, 512] bf16 DRAM, row 8*r+i = q[i, shard r].
                Returns lhsT tiles (q, -q^2/2), each [128, NT, 512] bf16."""
                ag = work.tile([128, NT, SHARD], bf, tag="ag")
                for r in range(NCORES):  # rank-block r -> partitions 32*(r%4)
                    eng = nc.sync if r % 2 == 0 else nc.scalar
                    eng.dma_start(
                        ag[32 * (r % 4) : 32 * (r % 4) + 8, r // 4, :],
                        src_ap[8 * r : 8 * r + 8, :],
                    )
                T1 = work.tile([128, NT, SHARD], bf, tag="T1")
                for t in range(NT):
                    nc.vector.transpose(T1[:, t, :], ag[:, t, :])
                T1h = work.tile([128, NT, SHARD], bf, tag="T1h")
                nc.vector.tensor_scalar_mul(T1h[:], T1[:], -0.5)
                T2 = work.tile([128, NT, SHARD], bf, tag="T2")
                nc.vector.tensor_mul(T2[:], T1[:], T1h[:])
                return [T1, T2]

            Ts = load_q(q_in[:])

            # --- SBUF-resident series matrices, derived on-chip from A1 ---
            # A1 ships as fp8(P*A_SCALE); the SWDGE DMA casts fp8->bf16 in
            # flight.  Everything stays scaled by lambda=A_SCALE:
            #   A1p = lambda*(P+P^2),  A2 = lambda*P^2
            # and the exp divides by lambda (ACT scale).  sq on ACT Square
            # (scale 1/sqrt(lambda) so (A1/sqrt(l))^2 = l*P^2); A1p on DVE.
            # The series' -1/2 factor lives in T2 = -q^2/2.
            A1 = wpool.tile([128, NCHUNK, SHARD], bf, tag="A1")
            A1p = wpool.tile([128, NCHUNK, SHARD], bf, tag="A1p")
            A2 = wpool.tile([128, NCHUNK, SHARD], bf, tag="A2")
            for g in range(NGRP):
                sl = slice(g * gsz, (g + 1) * gsz)
                nc.gpsimd.dma_start(A1[:, sl, :], A_in[:, sl, :])
                nc.scalar.activation(
                    A2[:, sl, :], A1[:, sl, :],
                    mybir.ActivationFunctionType.Square,
                    scale=1.0 / float(np.sqrt(A_SCALE)),
                )
                nc.vector.tensor_add(A1p[:, sl, :], A1[:, sl, :], A2[:, sl, :])
            D_sb = wpool.tile([B, SHARD], f32, tag="D")
            nc.sync.dma_start(D_sb[:], D_in[:])
            if COLTILE:
                sel_sb = wpool.tile([128, B], f32, tag="sel")
                nc.sync.dma_start(sel_sb[:], sel_in[:])

            for it in range(NITER):
                mats = [A1p, A2]
                if COLTILE:
                    # 4 concurrent accumulation chains in distinct PE column
                    # groups / PSUM banks; group g = v & 3 owns partitions
                    # [32g, 32g+8).  Reduced by a selector matmul afterwards.
                    pss = [
                        psum_pool.tile(
                            [128, SHARD], f32, tag=f"S{g}", bufs=1, name=f"ps{g}"
                        )
                        for g in range(4)
                    ]
                    seen = [0] * 4
                    order = [(k, v) for v in range(NCHUNK) for k in range(2)]
                    for k, v in order:
                        g = v & 3
                        t, off = v >> 4, (v & 15) * 32
                        nc.tensor.matmul(
                            pss[g][32 * g : 32 * g + B, :],
                            Ts[k][:, t, off : off + 8],
                            mats[k][:, v, :],
                            start=(seen[g] == 0),
                            stop=(seen[g] == 2 * (NCHUNK // 4) - 1),
                            tile_position=(0, 32 * g),
                        )
                        seen[g] += 1
                    Spart = work.tile([128, SHARD], f32, tag="Spart")
                    for g in range(4):
                        if g % 2 == 0:
                            nc.vector.tensor_copy(
                                Spart[32 * g : 32 * g + B, :],
                                pss[g][32 * g : 32 * g + B, :],
                            )
                        else:
                            nc.scalar.copy(
                                Spart[32 * g : 32 * g + B, :],
                                pss[g][32 * g : 32 * g + B, :],
                            )
                    ps = psum_pool.tile([B, SHARD], f32, tag="S")
                    nc.tensor.matmul(ps[:], sel_sb[:], Spart[:], start=True, stop=True)
                else:
                    ps = psum_pool.tile([B, SHARD], f32, tag="S")
                    n_mm = 2 * NCHUNK
                    mm = 0
                    for k in range(2):
                        for v in range(NCHUNK):
                            t, off = v >> 4, (v & 15) * 32
                            nc.tensor.matmul(
                                ps[:],
                                Ts[k][:, t, off : off + 8],
                                mats[k][:, v, :],
                                start=(mm == 0),
                                stop=(mm == n_mm - 1),
                            )
                            mm += 1

                qe = work.tile([B, SHARD], f32, tag="qe")
                nc.scalar.activation(
                    qe[:], ps[:], mybir.ActivationFunctionType.Exp,
                    scale=1.0 / A_SCALE,
                )
                if it == NITER - 1:
                    qf = work.tile([B, SHARD], f32, tag="qf")
                    nc.vector.tensor_mul(qf[:], qe[:], D_sb[:])
                    o = work.tile([B, SHARD], f32, tag="o")
                    nc.vector.tensor_scalar(
                        o[:], qf[:], -1.0, 1.0,
                        mybir.AluOpType.mult, mybir.AluOpType.add,
                    )
                    nc.sync.dma_start(out[:], o[:])
                else:
                    qb = work.tile([B, SHARD], bf, tag="qb")
                    nc.vector.tensor_mul(qb[:], qe[:], D_sb[:])
                    b_in = dram.tile([B, SHARD], bf, tag="bin")
                    b_out = dram.tile([NCORES * B, SHARD], bf, tag="bout")
                    nc.sync.dma_start(b_in[:], qb[:])
                    nc.gpsimd.collective_compute(
                        "AllGather",
                        mybir.AluOpType.bypass,
                        replica_groups=[list(range(NCORES))],
                        ins=[b_in[:]],
                        outs=[b_out[:]],
                    )
                    Ts = load_q(b_out[:])
    nc.finalize()
    return nc


_cache = {}


def _build_runner():
    """Compile once; return a callable(concat_inputs: dict) -> out [8, 4096]."""
    import jax
    from jax.sharding import Mesh, PartitionSpec
    from jax.experimental.shard_map import shard_map
    from concourse import bass2jax

    nc = build_bass()
    bass2jax.install_neuronx_cc_hook()

    partition_name = nc.partition_id_tensor.name if nc.partition_id_tensor else None
    in_names, out_names, out_avals, zero_out_shapes = [], [], [], []
    for alloc in nc.m.functions[0].allocations:
        if not isinstance(alloc, mybir.MemoryLocationSet):
            continue
        name = alloc.memorylocations[0].name
        if alloc.kind == "ExternalInput":
            if name != partition_name:
                in_names.append(name)
        elif alloc.kind == "ExternalOutput":
            out_names.append(name)
            out_avals.append(
                jax.core.ShapedArray(tuple(alloc.tensor_shape), mybir.dt.np(alloc.dtype))
            )
            zero_out_shapes.append((tuple(alloc.tensor_shape), mybir.dt.np(alloc.dtype)))
    n_params = len(in_names)
    all_in_names = list(in_names) + out_names
    if partition_name is not None:
        all_in_names.append(partition_name)

    def _body(*args):
        operands = list(args)
        if partition_name is not None:
            operands.append(bass2jax.partition_id_tensor())
        outs = bass2jax._bass_exec_p.bind(
            *operands,
            out_avals=tuple(out_avals),
            in_names=tuple(all_in_names),
            out_names=tuple(out_names),
            lowering_input_output_aliases=(),
            sim_require_finite=True,
            sim_require_nnan=True,
            nc=nc,
        )
        return tuple(outs)

    devices = jax.devices()[:NCORES]
    mesh = Mesh(np.asarray(devices), ("core",))
    n_outs = len(out_names)
    sharded = jax.jit(
        shard_map(
            _body,
            mesh=mesh,
            in_specs=(PartitionSpec("core"),) * (n_params + n_outs),
            out_specs=(PartitionSpec("core"),) * n_outs,
            check_rep=False,
        ),
        donate_argnums=tuple(range(n_params, n_params + n_outs)),
        keep_unused=True,
    )

    def runner(concat_inputs):
        concat_in = [concat_inputs[name] for name in in_names]
        concat_zeros = [
            np.zeros((NCORES * s[0], *s[1:]), dt) for s, dt in zero_out_shapes
        ]
        out_arrs = sharded(*concat_in, *concat_zeros)
        # single output "out": [NCORES*8, 512] -> [8, 4096]
        o = np.asarray(out_arrs[out_names.index("out")])
        return np.ascontiguousarray(
            o.reshape(NCORES, B, SHARD).transpose(1, 0, 2).reshape(B, N)
        )

    return runner


def _prep_inputs(preds, prob_matrix, seed_idx):
    """Host-side: build the concatenated (axis0-sharded) input arrays."""
    P = np.asarray(prob_matrix, np.float32)
    preds = np.asarray(preds, np.float32)
    seed_idx = np.asarray(seed_idx)

    A1s = (P * A_SCALE).astype(FP8)
    # permuted rows, then per-core column slices, concatenated on axis 0
    A_perm = A1s[_b_index().reshape(-1), :].reshape(128, NCHUNK, N)
    A1_cat = np.ascontiguousarray(
        A_perm.reshape(128, NCHUNK, NCORES, SHARD).transpose(2, 0, 1, 3)
    ).reshape(NCORES * 128, NCHUNK, SHARD)

    # q0 in AllGather layout: row 8*r+i = 1 - preds[i, 512*r : 512*(r+1)]
    q0 = np.ascontiguousarray(
        (1.0 - preds).reshape(B, NCORES, SHARD).transpose(1, 0, 2)
    ).reshape(NCORES * B, SHARD).astype(BF16)
    q0_cat = np.tile(q0, (NCORES, 1))

    # D = exp(-colsum(P + P^2/2)) * (1 - seed_mask), from the quantized P the
    # device uses (keeps host/device series consistent)
    Pf = (A1s.astype(np.float32) / A_SCALE).astype(BF16).astype(np.float32)
    C = Pf.sum(axis=0, dtype=np.float32) + 0.5 * np.einsum("ba,ba->a", Pf, Pf)
    maskc = np.ones((B, N), np.float32)
    maskc[seed_idx[:, 0], seed_idx[:, 1]] = 0.0
    D = np.exp(-C).astype(np.float32)[None, :] * maskc
    D_cat = np.ascontiguousarray(
        D.reshape(B, NCORES, SHARD).transpose(1, 0, 2)
    ).reshape(NCORES * B, SHARD)

    out = {"A1": A1_cat, "q0": q0_cat, "D": D_cat}
    if COLTILE:
        sel = np.zeros((128, B), np.float32)
        for g in range(4):
            for i in range(B):
                sel[32 * g + i, i] = 1.0
        out["sel"] = np.tile(sel, (NCORES, 1))
    return out


def run(preds, prob_matrix, seed_idx):
    if "runner" not in _cache:
        _cache["runner"] = _build_runner()
    return _cache["runner"](_prep_inputs(preds, prob_matrix, seed_idx))


def run_prepped(concat_inputs):
    if "runner" not in _cache:
        _cache["runner"] = _build_runner()
    return _cache["runner"](concat_inputs)


def kernel(preds, prob_matrix, seed_idx):
    return run(preds, prob_matrix, seed_idx)



# revision 2
# speedup vs baseline: 1.1785x; 1.1785x over previous
# DiffusionPropagate Trainium2 Bass kernel (v3).
#
# Math: new_pred[i,a] = 1 - prod_b(1 - P[b,a]*pred[i,b]), seeds clamped to 1,
# iterated NITER times.  In the complement domain q = 1 - pred, with the
# 2-term log series log(1-x) = -(x + x^2/2) and the post-saturation fact
# q in {~0, ~1} (so q^2 ~ q), one iteration collapses to a SINGLE matmul:
#   q_new = exp(q @ A/lambda - C - BIG*seed),  A = lambda*(P + P@..P^2/2)
# (P^2 elementwise).  C = colsum(dequant(A))/lambda is host-derived from the
# *quantized* A so the quantization error cancels when q ~= const.  The seed
# clamp is folded into the same PSUM accumulation as one extra matmul whose
# stationary rows are [ones | per-seed batch one-hots] and whose moving rows
# are [-lambda*C | -lambda*BIG*node one-hots]: exp(s - BIG) == 0 exactly.
#
# Distribution (8 cores): tensor-parallel over output nodes a; per-iteration
# AllGather of the bf16 q shard.  The gathered [64, 512] buffer feeds the
# next iteration's stationary tiles through a single XBAR transpose-DMA
# (dma_start_transpose): out[p, d, k] = in[k, 128d + p], which is exactly
# the b-on-partitions chunk layout the PE wants -- no on-chip transposes,
# no permutations.  One fp8 A matrix stays SBUF-resident; per iteration the
# device runs 33 matmul pairs + 1 exp + 1 DMA + 1 AllGather + 1 transpose.
import numpy as np
import ml_dtypes

import concourse.mybir as mybir
import concourse.tile as tile
from concourse import bacc

NCORES = 8
B = 8
N = 4096
NITER = 4
NSEEDS = 80
SHARD = N // NCORES          # 512
NCHUNK = N // 128            # 32 contraction chunks
NEXTRA = NSEEDS + 1          # bias-matmul contraction rows
NG = 8                       # A-matrix DMA groups (pipeline with iter-1 mm)

BF16 = ml_dtypes.bfloat16
FP8 = ml_dtypes.float8_e4m3
A_SCALE = 1024.0             # keeps fp8e4m3 entries of A in the normal range
BIG = 1024.0 * 1024.0        # -lambda*BIG/lambda = -1024 in the exponent


def build_bass():
    nc = bacc.Bacc(num_devices=NCORES)
    bf = mybir.dt.bfloat16
    f32 = mybir.dt.float32
    f8 = mybir.dt.float8e4

    A_in = nc.dram_tensor("A1", [128, NCHUNK, SHARD], f8, kind="ExternalInput")
    q_in = nc.dram_tensor("q0", [NCORES * B, SHARD], bf, kind="ExternalInput")
    sx_in = nc.dram_tensor("sext", [NEXTRA, B], bf, kind="ExternalInput")
    bm_in = nc.dram_tensor("bmv", [NEXTRA, SHARD], bf, kind="ExternalInput")
    out = nc.dram_tensor("out", [B, SHARD], f32, kind="ExternalOutput")

    gsz = NCHUNK // NG
    with tile.TileContext(nc) as tc:
        with (
            tc.tile_pool(name="weights", bufs=1) as wpool,
            tc.tile_pool(name="work", bufs=2) as work,
            tc.tile_pool(name="psum", bufs=2, space="PSUM") as psum_pool,
            tc.tile_pool(name="dram", bufs=NITER - 1, space="DRAM") as dram,
        ):
            A1 = wpool.tile([128, NCHUNK, SHARD], f8, tag="A1")
            for g in range(NG):
                sl = slice(g * gsz, (g + 1) * gsz)
                nc.gpsimd.dma_start(A1[:, sl, :], A_in[:, sl, :])
            sext = wpool.tile([NEXTRA, B], bf, tag="sext")
            nc.scalar.dma_start(sext[:], sx_in[:])
            bmv = wpool.tile([NEXTRA, SHARD], bf, tag="bmv")
            nc.scalar.dma_start(bmv[:], bm_in[:])
            # pre-warm the ACT Exp table off the critical path
            warm = work.tile([1, B], f32, tag="warm", bufs=1)
            nc.scalar.activation(
                warm[:], sext[0:1, :], mybir.ActivationFunctionType.Exp,
                scale=1.0 / A_SCALE,
            )

            src = q_in
            for it in range(NITER):
                # gathered [64, 512] -> stationary layout [128, 4, 64]:
                # T[p, d, 8r+i] = q[b, i] for b = 512r + 128d + p
                T = work.tile([128, 4, NCORES * B], bf, tag="T")
                nc.sync.dma_start_transpose(T[:], src[:])

                ps = psum_pool.tile([B, SHARD], f32, tag="S")
                for c in range(NCHUNK):
                    nc.tensor.matmul(
                        ps[:],
                        T[:, c & 3, 8 * (c >> 2) : 8 * (c >> 2) + 8],
                        A1[:, c, :],
                        start=(c == 0),
                        stop=False,
                    )
                nc.tensor.matmul(ps[:], sext[:], bmv[:], start=False, stop=True)

                if it < NITER - 1:
                    qb = work.tile([B, SHARD], bf, tag="qb")
                    nc.scalar.activation(
                        qb[:], ps[:], mybir.ActivationFunctionType.Exp,
                        scale=1.0 / A_SCALE,
                    )
                    b_in = dram.tile([B, SHARD], bf, tag="bin")
                    nc.sync.dma_start(b_in[:], qb[:])
                    b_out = dram.tile([NCORES * B, SHARD], bf, tag="bout")
                    nc.gpsimd.collective_compute(
                        "AllGather",
                        mybir.AluOpType.bypass,
                        replica_groups=[list(range(NCORES))],
                        ins=[b_in[:]],
                        outs=[b_out[:]],
                    )
                    src = b_out
                else:
                    o = work.tile([B, SHARD], f32, tag="o")
                    nc.scalar.activation(
                        o[:], ps[:], mybir.ActivationFunctionType.Exp,
                        scale=1.0 / A_SCALE,
                    )
                    nc.sync.dma_start(out[:], o[:])
    nc.finalize()
    return nc


_cache = {}


def _build_runner():
    """Compile once; return a callable(concat_inputs: dict) -> out [8, 4096]."""
    import jax
    from jax.sharding import Mesh, PartitionSpec
    from jax.experimental.shard_map import shard_map
    from concourse import bass2jax

    nc = build_bass()
    bass2jax.install_neuronx_cc_hook()

    partition_name = nc.partition_id_tensor.name if nc.partition_id_tensor else None
    in_names, out_names, out_avals, zero_out_shapes = [], [], [], []
    for alloc in nc.m.functions[0].allocations:
        if not isinstance(alloc, mybir.MemoryLocationSet):
            continue
        name = alloc.memorylocations[0].name
        if alloc.kind == "ExternalInput":
            if name != partition_name:
                in_names.append(name)
        elif alloc.kind == "ExternalOutput":
            out_names.append(name)
            out_avals.append(
                jax.core.ShapedArray(tuple(alloc.tensor_shape), mybir.dt.np(alloc.dtype))
            )
            zero_out_shapes.append((tuple(alloc.tensor_shape), mybir.dt.np(alloc.dtype)))
    n_params = len(in_names)
    all_in_names = list(in_names) + out_names
    if partition_name is not None:
        all_in_names.append(partition_name)

    def _body(*args):
        operands = list(args)
        if partition_name is not None:
            operands.append(bass2jax.partition_id_tensor())
        outs = bass2jax._bass_exec_p.bind(
            *operands,
            out_avals=tuple(out_avals),
            in_names=tuple(all_in_names),
            out_names=tuple(out_names),
            lowering_input_output_aliases=(),
            sim_require_finite=True,
            sim_require_nnan=True,
            nc=nc,
        )
        return tuple(outs)

    devices = jax.devices()[:NCORES]
    mesh = Mesh(np.asarray(devices), ("core",))
    n_outs = len(out_names)
    sharded = jax.jit(
        shard_map(
            _body,
            mesh=mesh,
            in_specs=(PartitionSpec("core"),) * (n_params + n_outs),
            out_specs=(PartitionSpec("core"),) * n_outs,
            check_rep=False,
        ),
        donate_argnums=tuple(range(n_params, n_params + n_outs)),
        keep_unused=True,
    )

    def runner(concat_inputs):
        concat_in = [concat_inputs[name] for name in in_names]
        concat_zeros = [
            np.zeros((NCORES * s[0], *s[1:]), dt) for s, dt in zero_out_shapes
        ]
        out_arrs = sharded(*concat_in, *concat_zeros)
        # single output "out": [NCORES*8, 512] of q4 -> preds [8, 4096]
        o = np.asarray(out_arrs[out_names.index("out")])
        q4 = np.ascontiguousarray(
            o.reshape(NCORES, B, SHARD).transpose(1, 0, 2).reshape(B, N)
        )
        return 1.0 - q4

    return runner


def _prep_inputs(preds, prob_matrix, seed_idx):
    """Host-side: build the concatenated (axis0-sharded) input arrays."""
    P = np.asarray(prob_matrix, np.float32)
    preds = np.asarray(preds, np.float32)
    seed_idx = np.asarray(seed_idx)

    # single series matrix, fp8, chunk layout A1[p, c, :] = A[128c + p, :]
    A = (P + 0.5 * P * P) * A_SCALE
    A8 = A.astype(FP8)
    A1 = np.ascontiguousarray(A8.reshape(NCHUNK, 128, N).transpose(1, 0, 2))
    A1_cat = np.ascontiguousarray(
        A1.reshape(128, NCHUNK, NCORES, SHARD).transpose(2, 0, 1, 3)
    ).reshape(NCORES * 128, NCHUNK, SHARD)

    # q0 in AllGather layout: row 8*r+i = 1 - preds[i, 512*r : 512*(r+1)]
    q0 = np.ascontiguousarray(
        (1.0 - preds).reshape(B, NCORES, SHARD).transpose(1, 0, 2)
    ).reshape(NCORES * B, SHARD).astype(BF16)
    q0_cat = np.tile(q0, (NCORES, 1))

    # bias matmul: row 0 carries -lambda*C (C from the dequantized A so the
    # fp8 error cancels when q ~= const); rows 1.. carry the seed clamps.
    C = A8.astype(np.float32).sum(axis=0)  # = lambda * colsum
    sext = np.zeros((NEXTRA, B), np.float32)
    sext[0, :] = 1.0
    bmv_full = np.zeros((NSEEDS, N), np.float32)
    for k in range(NSEEDS):
        sext[1 + k, seed_idx[k, 0]] = 1.0
        bmv_full[k, seed_idx[k, 1]] = -A_SCALE * BIG
    sext_cat = np.tile(sext.astype(BF16), (NCORES, 1))
    bmv = np.concatenate([-C[None, :], bmv_full], axis=0)
    bmv_cat = np.ascontiguousarray(
        bmv.reshape(NEXTRA, NCORES, SHARD).transpose(1, 0, 2)
    ).reshape(NCORES * NEXTRA, SHARD).astype(BF16)

    return {"A1": A1_cat, "q0": q0_cat, "sext": sext_cat, "bmv": bmv_cat}


def run(preds, prob_matrix, seed_idx):
    if "runner" not in _cache:
        _cache["runner"] = _build_runner()
    return _cache["runner"](_prep_inputs(preds, prob_matrix, seed_idx))


def run_prepped(concat_inputs):
    if "runner" not in _cache:
        _cache["runner"] = _build_runner()
    return _cache["runner"](concat_inputs)


def kernel(preds, prob_matrix, seed_idx):
    return run(preds, prob_matrix, seed_idx)


# revision 8
# speedup vs baseline: 1.8863x; 1.6006x over previous
# DiffusionPropagate Trainium2 Bass kernel (v6).
#
# Math: new_pred[i,a] = 1 - prod_b(1 - P[b,a]*pred[i,b]), seeds clamped to 1,
# iterated NITER times.  In the complement domain q = 1 - pred, with the
# 2-term log series log(1-x) = -(x + x^2/2) and the post-saturation fact
# q in {~0, ~1} (so q^2 ~ q), one iteration collapses to a SINGLE matmul:
#   q_new = exp(q @ A/lambda - C - BIG*seed),  A = lambda*(P + P^2/2)
# (P^2 elementwise).  C = colsum(dequant(A))/lambda is host-derived from the
# *quantized* A so the quantization error cancels when q ~= const.  The seed
# clamp is folded into the same PSUM accumulation as one extra matmul whose
# stationary rows are [-lambda*C | -lambda*BIG*node one-hots] and moving
# rows are [ones | per-seed batch one-hots]: exp(s - BIG) == 0 exactly.
#
# Distribution (8 cores): tensor-parallel over output nodes a; per-iteration
# AllGather of the bf16 q shard in node-major [512, 8] layout.
#
# Orientation: A is the STATIONARY operand ([128 b, 128 a] fp8 tiles, 32
# chunks x 4 a-tiles) and the gathered q chunks [128 b, 8 batch] are the
# moving operand, so the PSUM result is transposed: psT[a-part, batch].
# In the cost model Ldweights is free and matmul cost scales with the
# moving free size (8), so the whole 132-matmul phase takes ~1us and is
# insensitive to the PE p-state ramp; exp works on [128, 32] (tiny), and
# the shard DMAs in/out of the collective buffers are pure layout moves.
import numpy as np
import ml_dtypes

import concourse.mybir as mybir
import concourse.tile as tile
from concourse import bacc

NCORES = 8
B = 8
N = 4096
NITER = 4
NSEEDS = 80
SHARD = N // NCORES          # 512
NCHUNK = N // 128            # 32 contraction chunks
NEXTRA = NSEEDS + 1          # bias-matmul contraction rows
NG = 8                       # A-matrix DMA groups (pipeline with iter-1 mm)

BF16 = ml_dtypes.bfloat16
FP8 = ml_dtypes.float8_e4m3
A_SCALE = 1024.0             # keeps fp8e4m3 entries of A in the normal range
BIG = 1024.0 * 1024.0        # -lambda*BIG/lambda = -1024 in the exponent


def build_bass():
    nc = bacc.Bacc(num_devices=NCORES)
    bf = mybir.dt.bfloat16
    f32 = mybir.dt.float32
    f8 = mybir.dt.float8e4

    A_in = nc.dram_tensor("A1", [128, NCHUNK, SHARD], f8, kind="ExternalInput")
    q_in = nc.dram_tensor("q0", [128, NCHUNK, B], bf, kind="ExternalInput")
    sx_in = nc.dram_tensor("sext", [NEXTRA, B], bf, kind="ExternalInput")
    bm_in = nc.dram_tensor("bmv", [NEXTRA, SHARD], bf, kind="ExternalInput")
    out = nc.dram_tensor("out", [SHARD, B], f32, kind="ExternalOutput")

    gsz = NCHUNK // NG
    with tile.TileContext(nc) as tc:
        with (
            tc.tile_pool(name="weights", bufs=1) as wpool,
            tc.tile_pool(name="work", bufs=2) as work,
            tc.tile_pool(name="psum", bufs=2, space="PSUM") as psum_pool,
            tc.tile_pool(name="dram", bufs=NITER - 1, space="DRAM") as dram,
        ):
            sext = wpool.tile([NEXTRA, B], bf, tag="sext")
            nc.scalar.dma_start(sext[:], sx_in[:])
            bmv = wpool.tile([NEXTRA, SHARD], bf, tag="bmv")
            nc.scalar.dma_start(bmv[:], bm_in[:])
            A1 = wpool.tile([128, NCHUNK, SHARD], f8, tag="A1")
            for g in range(NG):
                sl = slice(g * gsz, (g + 1) * gsz)
                nc.gpsimd.dma_start(A1[:, sl, :], A_in[:, sl, :])
            # pre-warm the ACT Exp table off the critical path
            warm = work.tile([1, B], f32, tag="warm", bufs=1)
            nc.scalar.activation(
                warm[:], sext[0:1, :], mybir.ActivationFunctionType.Exp,
                scale=1.0 / A_SCALE,
            )

            src = None
            for it in range(NITER):
                # T[p, c, i] = q[b, i] for b = 128c + p (natural chunks)
                T = work.tile([128, NCHUNK, B], bf, tag="T")
                if it == 0:
                    nc.sync.dma_start(T[:], q_in[:])
                else:
                    nc.sync.dma_start(
                        T[:], src[:].rearrange("(m p) i -> p m i", p=128)
                    )

                # psT[a-local-part, g, batch] accumulated over b chunks;
                # chunk-major order so iter-1 consumes A DMA groups in order
                psT = psum_pool.tile([128, 4, B], f32, tag="S")
                for c in range(NCHUNK):
                    mv = T[:, c, :]
                    for g in range(4):
                        nc.tensor.matmul(
                            psT[:, g, :],
                            A1[:, c, 128 * g : 128 * g + 128],
                            mv,
                            start=(c == 0),
                            stop=False,
                        )
                for g in range(4):
                    nc.tensor.matmul(
                        psT[:, g, :], bmv[:, 128 * g : 128 * g + 128], sext[:],
                        start=False, stop=True,
                    )

                if it < NITER - 1:
                    qb = work.tile([128, 4, B], bf, tag="qb")
                    nc.scalar.activation(
                        qb[:], psT[:], mybir.ActivationFunctionType.Exp,
                        scale=1.0 / A_SCALE,
                    )
                    b_in = dram.tile([SHARD, B], bf, tag="bin")
                    nc.sync.dma_start(
                        b_in[:].rearrange("(g p) i -> p g i", g=4), qb[:]
                    )
                    b_out = dram.tile([NCORES * SHARD, B], bf, tag="bout")
                    nc.gpsimd.collective_compute(
                        "AllGather",
                        mybir.AluOpType.bypass,
                        replica_groups=[list(range(NCORES))],
                        ins=[b_in[:]],
                        outs=[b_out[:]],
                    )
                    src = b_out
                else:
                    o = work.tile([128, 4, B], f32, tag="o")
                    nc.scalar.activation(
                        o[:], psT[:], mybir.ActivationFunctionType.Exp,
                        scale=1.0 / A_SCALE,
                    )
                    nc.sync.dma_start(
                        out[:].rearrange("(g p) i -> p g i", g=4), o[:]
                    )
    nc.finalize()
    return nc


_cache = {}


def _build_runner():
    """Compile once; return a callable(concat_inputs: dict) -> out [8, 4096]."""
    import jax
    from jax.sharding import Mesh, PartitionSpec
    from jax.experimental.shard_map import shard_map
    from concourse import bass2jax

    nc = build_bass()
    bass2jax.install_neuronx_cc_hook()

    partition_name = nc.partition_id_tensor.name if nc.partition_id_tensor else None
    in_names, out_names, out_avals, zero_out_shapes = [], [], [], []
    for alloc in nc.m.functions[0].allocations:
        if not isinstance(alloc, mybir.MemoryLocationSet):
            continue
        name = alloc.memorylocations[0].name
        if alloc.kind == "ExternalInput":
            if name != partition_name:
                in_names.append(name)
        elif alloc.kind == "ExternalOutput":
            out_names.append(name)
            out_avals.append(
                jax.core.ShapedArray(tuple(alloc.tensor_shape), mybir.dt.np(alloc.dtype))
            )
            zero_out_shapes.append((tuple(alloc.tensor_shape), mybir.dt.np(alloc.dtype)))
    n_params = len(in_names)
    all_in_names = list(in_names) + out_names
    if partition_name is not None:
        all_in_names.append(partition_name)

    def _body(*args):
        operands = list(args)
        if partition_name is not None:
            operands.append(bass2jax.partition_id_tensor())
        outs = bass2jax._bass_exec_p.bind(
            *operands,
            out_avals=tuple(out_avals),
            in_names=tuple(all_in_names),
            out_names=tuple(out_names),
            lowering_input_output_aliases=(),
            sim_require_finite=True,
            sim_require_nnan=True,
            nc=nc,
        )
        return tuple(outs)

    devices = jax.devices()[:NCORES]
    mesh = Mesh(np.asarray(devices), ("core",))
    n_outs = len(out_names)
    sharded = jax.jit(
        shard_map(
            _body,
            mesh=mesh,
            in_specs=(PartitionSpec("core"),) * (n_params + n_outs),
            out_specs=(PartitionSpec("core"),) * n_outs,
            check_rep=False,
        ),
        donate_argnums=tuple(range(n_params, n_params + n_outs)),
        keep_unused=True,
    )

    def runner(concat_inputs):
        concat_in = [concat_inputs[name] for name in in_names]
        concat_zeros = [
            np.zeros((NCORES * s[0], *s[1:]), dt) for s, dt in zero_out_shapes
        ]
        out_arrs = sharded(*concat_in, *concat_zeros)
        # single output "out": [NCORES*512, 8] = q4 transposed, node-major
        o = np.asarray(out_arrs[out_names.index("out")])
        q4 = o.reshape(N, B).T
        return 1.0 - q4

    return runner


def _prep_inputs(preds, prob_matrix, seed_idx):
    """Host-side: build the concatenated (axis0-sharded) input arrays."""
    P = np.asarray(prob_matrix, np.float32)
    preds = np.asarray(preds, np.float32)
    seed_idx = np.asarray(seed_idx)

    # single series matrix, fp8, chunk layout A1[p, c, :] = A[128c + p, :]
    A = (P + 0.5 * P * P) * A_SCALE
    A8 = A.astype(FP8)
    A1 = np.ascontiguousarray(A8.reshape(NCHUNK, 128, N).transpose(1, 0, 2))
    A1_cat = np.ascontiguousarray(
        A1.reshape(128, NCHUNK, NCORES, SHARD).transpose(2, 0, 1, 3)
    ).reshape(NCORES * 128, NCHUNK, SHARD)

    # q0 directly in T layout: T[p, c, i] = q0[i, 128c + p]
    q0 = (1.0 - preds).astype(BF16)  # [B, N]
    q0T = np.ascontiguousarray(q0.T.reshape(NCHUNK, 128, B).transpose(1, 0, 2))
    q0_cat = np.tile(q0T, (NCORES, 1, 1))

    # bias matmul: stationary row 0 carries -lambda*C (C from the dequantized
    # A so the fp8 error cancels when q ~= const); rows 1.. the seed clamps.
    C = A8.astype(np.float32).sum(axis=0)  # = lambda * colsum
    sext = np.zeros((NEXTRA, B), np.float32)
    sext[0, :] = 1.0
    bmv_full = np.zeros((NSEEDS, N), np.float32)
    for kk in range(NSEEDS):
        sext[1 + kk, seed_idx[kk, 0]] = 1.0
        bmv_full[kk, seed_idx[kk, 1]] = -A_SCALE * BIG
    sext_cat = np.tile(sext.astype(BF16), (NCORES, 1))
    bmv = np.concatenate([-C[None, :], bmv_full], axis=0)
    bmv_cat = np.ascontiguousarray(
        bmv.reshape(NEXTRA, NCORES, SHARD).transpose(1, 0, 2)
    ).reshape(NCORES * NEXTRA, SHARD).astype(BF16)

    return {"A1": A1_cat, "q0": q0_cat, "sext": sext_cat, "bmv": bmv_cat}


def run(preds, prob_matrix, seed_idx):
    if "runner" not in _cache:
        _cache["runner"] = _build_runner()
    return _cache["runner"](_prep_inputs(preds, prob_matrix, seed_idx))


def run_prepped(concat_inputs):
    if "runner" not in _cache:
        _cache["runner"] = _build_runner()
    return _cache["runner"](concat_inputs)


def kernel(preds, prob_matrix, seed_idx):
    return run(preds, prob_matrix, seed_idx)


# revision 13
# speedup vs baseline: 1.9248x; 1.0204x over previous
# DiffusionPropagate Trainium2 Bass kernel (v6).
#
# Math: new_pred[i,a] = 1 - prod_b(1 - P[b,a]*pred[i,b]), seeds clamped to 1,
# iterated NITER times.  In the complement domain q = 1 - pred, with the
# 2-term log series log(1-x) = -(x + x^2/2) and the post-saturation fact
# q in {~0, ~1} (so q^2 ~ q), one iteration collapses to a SINGLE matmul:
#   q_new = exp(q @ A/lambda - C - BIG*seed),  A = lambda*(P + P^2/2)
# (P^2 elementwise).  C = colsum(dequant(A))/lambda is host-derived from the
# *quantized* A so the quantization error cancels when q ~= const.  The seed
# clamp is folded into the same PSUM accumulation as one extra matmul whose
# stationary rows are [-lambda*C | -lambda*BIG*node one-hots] and moving
# rows are [ones | per-seed batch one-hots]: exp(s - BIG) == 0 exactly.
#
# Distribution (8 cores): tensor-parallel over output nodes a; per-iteration
# AllGather of the bf16 q shard in node-major [512, 8] layout.
#
# Orientation: A is the STATIONARY operand ([128 b, 128 a] fp8 tiles, 32
# chunks x 4 a-tiles) and the gathered q chunks [128 b, 8 batch] are the
# moving operand, so the PSUM result is transposed: psT[a-part, batch].
# In the cost model Ldweights is free and matmul cost scales with the
# moving free size (8), so the whole 132-matmul phase takes ~1us and is
# insensitive to the PE p-state ramp; exp works on [128, 32] (tiny), and
# the shard DMAs in/out of the collective buffers are pure layout moves.
import numpy as np
import ml_dtypes

import concourse.mybir as mybir
import concourse.tile as tile
from concourse import bacc

NCORES = 8
B = 8
N = 4096
NITER = 4
NSEEDS = 80
SHARD = N // NCORES          # 512
NCHUNK = N // 128            # 32 contraction chunks
NEXTRA = NSEEDS + 1          # bias-matmul contraction rows
NG = 8                       # A-matrix DMA groups (pipeline with iter-1 mm)

BF16 = ml_dtypes.bfloat16
FP8 = ml_dtypes.float8_e4m3
A_SCALE = 1024.0             # keeps fp8e4m3 entries of A in the normal range
BIG = 1024.0 * 1024.0        # -lambda*BIG/lambda = -1024 in the exponent


def build_bass():
    nc = bacc.Bacc(num_devices=NCORES)
    bf = mybir.dt.bfloat16
    f32 = mybir.dt.float32
    f8 = mybir.dt.float8e4

    A_in = nc.dram_tensor("A1", [128, NCHUNK, SHARD], f8, kind="ExternalInput")
    q_in = nc.dram_tensor("q0", [128, NCHUNK, B], bf, kind="ExternalInput")
    sx_in = nc.dram_tensor("sext", [NEXTRA, B], bf, kind="ExternalInput")
    bm_in = nc.dram_tensor("bmv", [NEXTRA, SHARD], bf, kind="ExternalInput")
    out = nc.dram_tensor("out", [SHARD, B], bf, kind="ExternalOutput")

    gsz = NCHUNK // NG
    with tile.TileContext(nc) as tc:
        with (
            tc.tile_pool(name="weights", bufs=1) as wpool,
            tc.tile_pool(name="work", bufs=2) as work,
            tc.tile_pool(name="psum", bufs=2, space="PSUM") as psum_pool,
            tc.tile_pool(name="dram", bufs=NITER - 1, space="DRAM") as dram,
        ):
            sext = wpool.tile([NEXTRA, B], bf, tag="sext")
            nc.scalar.dma_start(sext[:], sx_in[:])
            bmv = wpool.tile([NEXTRA, SHARD], bf, tag="bmv")
            nc.scalar.dma_start(bmv[:], bm_in[:])
            A1 = wpool.tile([128, NCHUNK, SHARD], f8, tag="A1")
            for g in range(NG):
                sl = slice(g * gsz, (g + 1) * gsz)
                eng = nc.gpsimd if g % 2 == 0 else nc.scalar
                eng.dma_start(A1[:, sl, :], A_in[:, sl, :])
            # pre-warm the ACT Exp table off the critical path
            warm = work.tile([1, B], f32, tag="warm", bufs=1)
            nc.scalar.activation(
                warm[:], sext[0:1, :], mybir.ActivationFunctionType.Exp,
                scale=1.0 / A_SCALE,
            )

            src = None
            for it in range(NITER):
                # T[p, c, i] = q[b, i] for b = 128c + p (natural chunks)
                T = work.tile([128, NCHUNK, B], bf, tag="T")
                if it == 0:
                    nc.sync.dma_start(T[:], q_in[:])
                else:
                    nc.sync.dma_start(
                        T[:], src[:].rearrange("(m p) i -> p m i", p=128)
                    )

                # psT[a-local-part, g, batch] accumulated over b chunks;
                # chunk-major order so iter-1 consumes A DMA groups in order
                psT = psum_pool.tile([128, 4, B], f32, tag="S")
                for c in range(NCHUNK):
                    mv = T[:, c, :]
                    for g in range(4):
                        nc.tensor.matmul(
                            psT[:, g, :],
                            A1[:, c, 128 * g : 128 * g + 128],
                            mv,
                            start=(c == 0),
                            stop=False,
                        )
                for g in range(4):
                    nc.tensor.matmul(
                        psT[:, g, :], bmv[:, 128 * g : 128 * g + 128], sext[:],
                        start=False, stop=True,
                    )

                if it < NITER - 1:
                    qb = work.tile([128, 4, B], bf, tag="qb")
                    nc.scalar.activation(
                        qb[:], psT[:], mybir.ActivationFunctionType.Exp,
                        scale=1.0 / A_SCALE,
                    )
                    b_in = dram.tile([SHARD, B], bf, tag="bin")
                    nc.sync.dma_start(
                        b_in[:].rearrange("(g p) i -> p g i", g=4), qb[:]
                    )
                    b_out = dram.tile([NCORES * SHARD, B], bf, tag="bout")
                    nc.gpsimd.collective_compute(
                        "AllGather",
                        mybir.AluOpType.bypass,
                        replica_groups=[list(range(NCORES))],
                        ins=[b_in[:]],
                        outs=[b_out[:]],
                    )
                    src = b_out
                else:
                    o = work.tile([128, 4, B], bf, tag="o")
                    nc.scalar.activation(
                        o[:], psT[:], mybir.ActivationFunctionType.Exp,
                        scale=1.0 / A_SCALE,
                    )
                    nc.sync.dma_start(
                        out[:].rearrange("(g p) i -> p g i", g=4), o[:]
                    )
    nc.finalize()
    return nc


_cache = {}


def _build_runner():
    """Compile once; return a callable(concat_inputs: dict) -> out [8, 4096]."""
    import jax
    from jax.sharding import Mesh, PartitionSpec
    from jax.experimental.shard_map import shard_map
    from concourse import bass2jax

    nc = build_bass()
    bass2jax.install_neuronx_cc_hook()

    partition_name = nc.partition_id_tensor.name if nc.partition_id_tensor else None
    in_names, out_names, out_avals, zero_out_shapes = [], [], [], []
    for alloc in nc.m.functions[0].allocations:
        if not isinstance(alloc, mybir.MemoryLocationSet):
            continue
        name = alloc.memorylocations[0].name
        if alloc.kind == "ExternalInput":
            if name != partition_name:
                in_names.append(name)
        elif alloc.kind == "ExternalOutput":
            out_names.append(name)
            out_avals.append(
                jax.core.ShapedArray(tuple(alloc.tensor_shape), mybir.dt.np(alloc.dtype))
            )
            zero_out_shapes.append((tuple(alloc.tensor_shape), mybir.dt.np(alloc.dtype)))
    n_params = len(in_names)
    all_in_names = list(in_names) + out_names
    if partition_name is not None:
        all_in_names.append(partition_name)

    def _body(*args):
        operands = list(args)
        if partition_name is not None:
            operands.append(bass2jax.partition_id_tensor())
        outs = bass2jax._bass_exec_p.bind(
            *operands,
            out_avals=tuple(out_avals),
            in_names=tuple(all_in_names),
            out_names=tuple(out_names),
            lowering_input_output_aliases=(),
            sim_require_finite=True,
            sim_require_nnan=True,
            nc=nc,
        )
        return tuple(outs)

    devices = jax.devices()[:NCORES]
    mesh = Mesh(np.asarray(devices), ("core",))
    n_outs = len(out_names)
    sharded = jax.jit(
        shard_map(
            _body,
            mesh=mesh,
            in_specs=(PartitionSpec("core"),) * (n_params + n_outs),
            out_specs=(PartitionSpec("core"),) * n_outs,
            check_rep=False,
        ),
        donate_argnums=tuple(range(n_params, n_params + n_outs)),
        keep_unused=True,
    )

    def runner(concat_inputs):
        concat_in = [concat_inputs[name] for name in in_names]
        concat_zeros = [
            np.zeros((NCORES * s[0], *s[1:]), dt) for s, dt in zero_out_shapes
        ]
        out_arrs = sharded(*concat_in, *concat_zeros)
        # single output "out": [NCORES*512, 8] = q4 transposed, node-major
        o = np.asarray(out_arrs[out_names.index("out")]).astype(np.float32)
        q4 = o.reshape(N, B).T
        return 1.0 - q4

    return runner


def _prep_inputs(preds, prob_matrix, seed_idx):
    """Host-side: build the concatenated (axis0-sharded) input arrays."""
    P = np.asarray(prob_matrix, np.float32)
    preds = np.asarray(preds, np.float32)
    seed_idx = np.asarray(seed_idx)

    # single series matrix, fp8, chunk layout A1[p, c, :] = A[128c + p, :]
    A = (P + 0.5 * P * P) * A_SCALE
    A8 = A.astype(FP8)
    A1 = np.ascontiguousarray(A8.reshape(NCHUNK, 128, N).transpose(1, 0, 2))
    A1_cat = np.ascontiguousarray(
        A1.reshape(128, NCHUNK, NCORES, SHARD).transpose(2, 0, 1, 3)
    ).reshape(NCORES * 128, NCHUNK, SHARD)

    # q0 directly in T layout: T[p, c, i] = q0[i, 128c + p]
    q0 = (1.0 - preds).astype(BF16)  # [B, N]
    q0T = np.ascontiguousarray(q0.T.reshape(NCHUNK, 128, B).transpose(1, 0, 2))
    q0_cat = np.tile(q0T, (NCORES, 1, 1))

    # bias matmul: stationary row 0 carries -lambda*C (C from the dequantized
    # A so the fp8 error cancels when q ~= const); rows 1.. the seed clamps.
    C = A8.astype(np.float32).sum(axis=0)  # = lambda * colsum
    sext = np.zeros((NEXTRA, B), np.float32)
    sext[0, :] = 1.0
    bmv_full = np.zeros((NSEEDS, N), np.float32)
    for kk in range(NSEEDS):
        sext[1 + kk, seed_idx[kk, 0]] = 1.0
        bmv_full[kk, seed_idx[kk, 1]] = -A_SCALE * BIG
    sext_cat = np.tile(sext.astype(BF16), (NCORES, 1))
    bmv = np.concatenate([-C[None, :], bmv_full], axis=0)
    bmv_cat = np.ascontiguousarray(
        bmv.reshape(NEXTRA, NCORES, SHARD).transpose(1, 0, 2)
    ).reshape(NCORES * NEXTRA, SHARD).astype(BF16)

    return {"A1": A1_cat, "q0": q0_cat, "sext": sext_cat, "bmv": bmv_cat}


def run(preds, prob_matrix, seed_idx):
    if "runner" not in _cache:
        _cache["runner"] = _build_runner()
    return _cache["runner"](_prep_inputs(preds, prob_matrix, seed_idx))


def run_prepped(concat_inputs):
    if "runner" not in _cache:
        _cache["runner"] = _build_runner()
    return _cache["runner"](concat_inputs)


def kernel(preds, prob_matrix, seed_idx):
    return run(preds, prob_matrix, seed_idx)


# revision 14
# speedup vs baseline: 1.9284x; 1.0019x over previous
# DiffusionPropagate Trainium2 Bass kernel (v6).
#
# Math: new_pred[i,a] = 1 - prod_b(1 - P[b,a]*pred[i,b]), seeds clamped to 1,
# iterated NITER times.  In the complement domain q = 1 - pred, with the
# 2-term log series log(1-x) = -(x + x^2/2) and the post-saturation fact
# q in {~0, ~1} (so q^2 ~ q), one iteration collapses to a SINGLE matmul:
#   q_new = exp(q @ A/lambda - C - BIG*seed),  A = lambda*(P + P^2/2)
# (P^2 elementwise).  C = colsum(dequant(A))/lambda is host-derived from the
# *quantized* A so the quantization error cancels when q ~= const.  The seed
# clamp is folded into the same PSUM accumulation as one extra matmul whose
# stationary rows are [-lambda*C | -lambda*BIG*node one-hots] and moving
# rows are [ones | per-seed batch one-hots]: exp(s - BIG) == 0 exactly.
#
# Distribution (8 cores): tensor-parallel over output nodes a; per-iteration
# AllGather of the bf16 q shard in node-major [512, 8] layout.
#
# Orientation: A is the STATIONARY operand ([128 b, 128 a] fp8 tiles, 32
# chunks x 4 a-tiles) and the gathered q chunks [128 b, 8 batch] are the
# moving operand, so the PSUM result is transposed: psT[a-part, batch].
# In the cost model Ldweights is free and matmul cost scales with the
# moving free size (8), so the whole 132-matmul phase takes ~1us and is
# insensitive to the PE p-state ramp; exp works on [128, 32] (tiny), and
# the shard DMAs in/out of the collective buffers are pure layout moves.
import numpy as np
import ml_dtypes

import concourse.mybir as mybir
import concourse.tile as tile
from concourse import bacc

NCORES = 8
B = 8
N = 4096
NITER = 4
NSEEDS = 80
SHARD = N // NCORES          # 512
NCHUNK = N // 128            # 32 contraction chunks
NEXTRA = NSEEDS + 1          # bias-matmul contraction rows
NG = 8                       # A-matrix DMA groups (pipeline with iter-1 mm)

BF16 = ml_dtypes.bfloat16
FP8 = ml_dtypes.float8_e4m3
A_SCALE = 1024.0             # keeps fp8e4m3 entries of A in the normal range
BIG = 1024.0 * 1024.0        # -lambda*BIG/lambda = -1024 in the exponent


def build_bass():
    nc = bacc.Bacc(num_devices=NCORES)
    bf = mybir.dt.bfloat16
    f32 = mybir.dt.float32
    f8 = mybir.dt.float8e4

    A_in = nc.dram_tensor("A1", [128, NCHUNK, SHARD], f8, kind="ExternalInput")
    q_in = nc.dram_tensor("q0", [128, NCHUNK, B], bf, kind="ExternalInput")
    sx_in = nc.dram_tensor("sext", [NEXTRA, B], bf, kind="ExternalInput")
    bm_in = nc.dram_tensor("bmv", [NEXTRA, SHARD], bf, kind="ExternalInput")
    out = nc.dram_tensor("out", [SHARD, B], bf, kind="ExternalOutput")

    gsz = NCHUNK // NG
    with tile.TileContext(nc) as tc:
        with (
            tc.tile_pool(name="weights", bufs=1) as wpool,
            tc.tile_pool(name="work", bufs=2) as work,
            tc.tile_pool(name="psum", bufs=2, space="PSUM") as psum_pool,
            tc.tile_pool(name="dram", bufs=NITER - 1, space="DRAM") as dram,
        ):
            sext = wpool.tile([NEXTRA, B], bf, tag="sext")
            nc.scalar.dma_start(sext[:], sx_in[:])
            bmv = wpool.tile([NEXTRA, SHARD], bf, tag="bmv")
            nc.scalar.dma_start(bmv[:], bm_in[:])
            A1 = wpool.tile([128, NCHUNK, SHARD], f8, tag="A1")
            for g in range(NG):
                sl = slice(g * gsz, (g + 1) * gsz)
                eng = nc.gpsimd if g % 2 == 0 else nc.scalar
                eng.dma_start(A1[:, sl, :], A_in[:, sl, :])
            # pre-warm the ACT Exp table off the critical path
            warm = work.tile([1, B], f32, tag="warm", bufs=1)
            nc.scalar.activation(
                warm[:], sext[0:1, :], mybir.ActivationFunctionType.Exp,
                scale=1.0 / A_SCALE,
            )

            src = None
            for it in range(NITER):
                # T[p, c, i] = q[b, i] for b = 128c + p (natural chunks)
                T = work.tile([128, NCHUNK, B], bf, tag="T")
                if it == 0:
                    nc.sync.dma_start(T[:], q_in[:])
                else:
                    half = src[:].rearrange("(m p) i -> p m i", p=128)
                    nc.sync.dma_start(T[:, 0:16, :], half[:, 0:16, :])
                    nc.scalar.dma_start(T[:, 16:32, :], half[:, 16:32, :])

                # psT[a-local-part, g, batch] accumulated over b chunks;
                # chunk-major order so iter-1 consumes A DMA groups in order
                psT = psum_pool.tile([128, 4, B], f32, tag="S")
                for c in range(NCHUNK):
                    mv = T[:, c, :]
                    for g in range(4):
                        nc.tensor.matmul(
                            psT[:, g, :],
                            A1[:, c, 128 * g : 128 * g + 128],
                            mv,
                            start=(c == 0),
                            stop=False,
                        )
                for g in range(4):
                    nc.tensor.matmul(
                        psT[:, g, :], bmv[:, 128 * g : 128 * g + 128], sext[:],
                        start=False, stop=True,
                    )

                if it < NITER - 1:
                    qb = work.tile([128, 4, B], bf, tag="qb")
                    nc.scalar.activation(
                        qb[:], psT[:], mybir.ActivationFunctionType.Exp,
                        scale=1.0 / A_SCALE,
                    )
                    b_in = dram.tile([SHARD, B], bf, tag="bin")
                    nc.sync.dma_start(
                        b_in[:].rearrange("(g p) i -> p g i", g=4), qb[:]
                    )
                    b_out = dram.tile([NCORES * SHARD, B], bf, tag="bout")
                    nc.gpsimd.collective_compute(
                        "AllGather",
                        mybir.AluOpType.bypass,
                        replica_groups=[list(range(NCORES))],
                        ins=[b_in[:]],
                        outs=[b_out[:]],
                    )
                    src = b_out
                else:
                    o = work.tile([128, 4, B], bf, tag="o")
                    nc.scalar.activation(
                        o[:], psT[:], mybir.ActivationFunctionType.Exp,
                        scale=1.0 / A_SCALE,
                    )
                    nc.sync.dma_start(
                        out[:].rearrange("(g p) i -> p g i", g=4), o[:]
                    )
    nc.finalize()
    return nc


_cache = {}


def _build_runner():
    """Compile once; return a callable(concat_inputs: dict) -> out [8, 4096]."""
    import jax
    from jax.sharding import Mesh, PartitionSpec
    from jax.experimental.shard_map import shard_map
    from concourse import bass2jax

    nc = build_bass()
    bass2jax.install_neuronx_cc_hook()

    partition_name = nc.partition_id_tensor.name if nc.partition_id_tensor else None
    in_names, out_names, out_avals, zero_out_shapes = [], [], [], []
    for alloc in nc.m.functions[0].allocations:
        if not isinstance(alloc, mybir.MemoryLocationSet):
            continue
        name = alloc.memorylocations[0].name
        if alloc.kind == "ExternalInput":
            if name != partition_name:
                in_names.append(name)
        elif alloc.kind == "ExternalOutput":
            out_names.append(name)
            out_avals.append(
                jax.core.ShapedArray(tuple(alloc.tensor_shape), mybir.dt.np(alloc.dtype))
            )
            zero_out_shapes.append((tuple(alloc.tensor_shape), mybir.dt.np(alloc.dtype)))
    n_params = len(in_names)
    all_in_names = list(in_names) + out_names
    if partition_name is not None:
        all_in_names.append(partition_name)

    def _body(*args):
        operands = list(args)
        if partition_name is not None:
            operands.append(bass2jax.partition_id_tensor())
        outs = bass2jax._bass_exec_p.bind(
            *operands,
            out_avals=tuple(out_avals),
            in_names=tuple(all_in_names),
            out_names=tuple(out_names),
            lowering_input_output_aliases=(),
            sim_require_finite=True,
            sim_require_nnan=True,
            nc=nc,
        )
        return tuple(outs)

    devices = jax.devices()[:NCORES]
    mesh = Mesh(np.asarray(devices), ("core",))
    n_outs = len(out_names)
    sharded = jax.jit(
        shard_map(
            _body,
            mesh=mesh,
            in_specs=(PartitionSpec("core"),) * (n_params + n_outs),
            out_specs=(PartitionSpec("core"),) * n_outs,
            check_rep=False,
        ),
        donate_argnums=tuple(range(n_params, n_params + n_outs)),
        keep_unused=True,
    )

    def runner(concat_inputs):
        concat_in = [concat_inputs[name] for name in in_names]
        concat_zeros = [
            np.zeros((NCORES * s[0], *s[1:]), dt) for s, dt in zero_out_shapes
        ]
        out_arrs = sharded(*concat_in, *concat_zeros)
        # single output "out": [NCORES*512, 8] = q4 transposed, node-major
        o = np.asarray(out_arrs[out_names.index("out")]).astype(np.float32)
        q4 = o.reshape(N, B).T
        return 1.0 - q4

    return runner


def _prep_inputs(preds, prob_matrix, seed_idx):
    """Host-side: build the concatenated (axis0-sharded) input arrays."""
    P = np.asarray(prob_matrix, np.float32)
    preds = np.asarray(preds, np.float32)
    seed_idx = np.asarray(seed_idx)

    # single series matrix, fp8, chunk layout A1[p, c, :] = A[128c + p, :]
    A = (P + 0.5 * P * P) * A_SCALE
    A8 = A.astype(FP8)
    A1 = np.ascontiguousarray(A8.reshape(NCHUNK, 128, N).transpose(1, 0, 2))
    A1_cat = np.ascontiguousarray(
        A1.reshape(128, NCHUNK, NCORES, SHARD).transpose(2, 0, 1, 3)
    ).reshape(NCORES * 128, NCHUNK, SHARD)

    # q0 directly in T layout: T[p, c, i] = q0[i, 128c + p]
    q0 = (1.0 - preds).astype(BF16)  # [B, N]
    q0T = np.ascontiguousarray(q0.T.reshape(NCHUNK, 128, B).transpose(1, 0, 2))
    q0_cat = np.tile(q0T, (NCORES, 1, 1))

    # bias matmul: stationary row 0 carries -lambda*C (C from the dequantized
    # A so the fp8 error cancels when q ~= const); rows 1.. the seed clamps.
    C = A8.astype(np.float32).sum(axis=0)  # = lambda * colsum
    sext = np.zeros((NEXTRA, B), np.float32)
    sext[0, :] = 1.0
    bmv_full = np.zeros((NSEEDS, N), np.float32)
    for kk in range(NSEEDS):
        sext[1 + kk, seed_idx[kk, 0]] = 1.0
        bmv_full[kk, seed_idx[kk, 1]] = -A_SCALE * BIG
    sext_cat = np.tile(sext.astype(BF16), (NCORES, 1))
    bmv = np.concatenate([-C[None, :], bmv_full], axis=0)
    bmv_cat = np.ascontiguousarray(
        bmv.reshape(NEXTRA, NCORES, SHARD).transpose(1, 0, 2)
    ).reshape(NCORES * NEXTRA, SHARD).astype(BF16)

    return {"A1": A1_cat, "q0": q0_cat, "sext": sext_cat, "bmv": bmv_cat}


def run(preds, prob_matrix, seed_idx):
    if "runner" not in _cache:
        _cache["runner"] = _build_runner()
    return _cache["runner"](_prep_inputs(preds, prob_matrix, seed_idx))


def run_prepped(concat_inputs):
    if "runner" not in _cache:
        _cache["runner"] = _build_runner()
    return _cache["runner"](concat_inputs)


def kernel(preds, prob_matrix, seed_idx):
    return run(preds, prob_matrix, seed_idx)


# revision 15
# speedup vs baseline: 1.9823x; 1.0280x over previous
# DiffusionPropagate Trainium2 Bass kernel (v6).
#
# Math: new_pred[i,a] = 1 - prod_b(1 - P[b,a]*pred[i,b]), seeds clamped to 1,
# iterated NITER times.  In the complement domain q = 1 - pred, with the
# 2-term log series log(1-x) = -(x + x^2/2) and the post-saturation fact
# q in {~0, ~1} (so q^2 ~ q), one iteration collapses to a SINGLE matmul:
#   q_new = exp(q @ A/lambda - C - BIG*seed),  A = lambda*(P + P^2/2)
# (P^2 elementwise).  C = colsum(dequant(A))/lambda is host-derived from the
# *quantized* A so the quantization error cancels when q ~= const.  The seed
# clamp is folded into the same PSUM accumulation as one extra matmul whose
# stationary rows are [-lambda*C | -lambda*BIG*node one-hots] and moving
# rows are [ones | per-seed batch one-hots]: exp(s - BIG) == 0 exactly.
#
# Distribution (8 cores): tensor-parallel over output nodes a; per-iteration
# AllGather of the bf16 q shard in node-major [512, 8] layout.
#
# Orientation: A is the STATIONARY operand ([128 b, 128 a] fp8 tiles, 32
# chunks x 4 a-tiles) and the gathered q chunks [128 b, 8 batch] are the
# moving operand, so the PSUM result is transposed: psT[a-part, batch].
# In the cost model Ldweights is free and matmul cost scales with the
# moving free size (8), so the whole 132-matmul phase takes ~1us and is
# insensitive to the PE p-state ramp; exp works on [128, 32] (tiny), and
# the shard DMAs in/out of the collective buffers are pure layout moves.
import numpy as np
import ml_dtypes

import concourse.mybir as mybir
import concourse.tile as tile
from concourse import bacc

NCORES = 8
B = 8
N = 4096
NITER = 4
NSEEDS = 80
SHARD = N // NCORES          # 512
NCHUNK = N // 128            # 32 contraction chunks
NEXTRA = NSEEDS + 1          # bias-matmul contraction rows
NG = 8                       # A-matrix DMA groups (pipeline with iter-1 mm)

BF16 = ml_dtypes.bfloat16
FP8 = ml_dtypes.float8_e4m3
A_SCALE = 1024.0             # keeps fp8e4m3 entries of A in the normal range
BIG = 1024.0 * 1024.0        # -lambda*BIG/lambda = -1024 in the exponent


def build_bass():
    nc = bacc.Bacc(num_devices=NCORES)
    bf = mybir.dt.bfloat16
    f32 = mybir.dt.float32
    f8 = mybir.dt.float8e4

    A_in = nc.dram_tensor("A1", [128, NCHUNK, SHARD], f8, kind="ExternalInput")
    q_in = nc.dram_tensor("q0", [128, NCHUNK, B], f8, kind="ExternalInput")
    sx_in = nc.dram_tensor("sext", [NEXTRA, B], bf, kind="ExternalInput")
    bm_in = nc.dram_tensor("bmv", [NEXTRA, SHARD], bf, kind="ExternalInput")
    out = nc.dram_tensor("out", [SHARD, B], bf, kind="ExternalOutput")

    gsz = NCHUNK // NG
    with tile.TileContext(nc) as tc:
        with (
            tc.tile_pool(name="weights", bufs=1) as wpool,
            tc.tile_pool(name="work", bufs=2) as work,
            tc.tile_pool(name="psum", bufs=2, space="PSUM") as psum_pool,
            tc.tile_pool(name="dram", bufs=NITER - 1, space="DRAM") as dram,
        ):
            sext = wpool.tile([NEXTRA, B], bf, tag="sext")
            nc.scalar.dma_start(sext[:], sx_in[:])
            bmv = wpool.tile([NEXTRA, SHARD], bf, tag="bmv")
            nc.scalar.dma_start(bmv[:], bm_in[:])
            A1 = wpool.tile([128, NCHUNK, SHARD], f8, tag="A1")
            for g in range(NG):
                sl = slice(g * gsz, (g + 1) * gsz)
                eng = nc.gpsimd if g % 2 == 0 else nc.scalar
                eng.dma_start(A1[:, sl, :], A_in[:, sl, :])
            # pre-warm the ACT Exp table off the critical path
            warm = work.tile([1, B], f32, tag="warm", bufs=1)
            nc.scalar.activation(
                warm[:], sext[0:1, :], mybir.ActivationFunctionType.Exp,
                scale=1.0 / A_SCALE,
            )

            src = None
            for it in range(NITER):
                # T[p, c, i] = q[b, i] for b = 128c + p (natural chunks)
                T = work.tile([128, NCHUNK, B], f8, tag="T")
                if it == 0:
                    nc.sync.dma_start(T[:], q_in[:])
                else:
                    half = src[:].rearrange("(m p) i -> p m i", p=128)
                    nc.sync.dma_start(T[:, 0:16, :], half[:, 0:16, :])
                    nc.scalar.dma_start(T[:, 16:32, :], half[:, 16:32, :])

                # psT[a-local-part, g, batch] accumulated over b chunks;
                # chunk-major order so iter-1 consumes A DMA groups in order
                psT = psum_pool.tile([128, 4, B], f32, tag="S")
                for c in range(NCHUNK):
                    mv = T[:, c, :]
                    for g in range(4):
                        nc.tensor.matmul(
                            psT[:, g, :],
                            A1[:, c, 128 * g : 128 * g + 128],
                            mv,
                            start=(c == 0),
                            stop=False,
                        )
                for g in range(4):
                    nc.tensor.matmul(
                        psT[:, g, :], bmv[:, 128 * g : 128 * g + 128], sext[:],
                        start=False, stop=True,
                    )

                if it < NITER - 1:
                    qb = work.tile([128, 4, B], f8, tag="qb")
                    nc.scalar.activation(
                        qb[:], psT[:], mybir.ActivationFunctionType.Exp,
                        scale=1.0 / A_SCALE,
                    )
                    b_in = dram.tile([SHARD, B], f8, tag="bin")
                    nc.sync.dma_start(
                        b_in[:].rearrange("(g p) i -> p g i", g=4), qb[:]
                    )
                    b_out = dram.tile([NCORES * SHARD, B], f8, tag="bout")
                    nc.gpsimd.collective_compute(
                        "AllGather",
                        mybir.AluOpType.bypass,
                        replica_groups=[list(range(NCORES))],
                        ins=[b_in[:]],
                        outs=[b_out[:]],
                    )
                    src = b_out
                else:
                    o = work.tile([128, 4, B], bf, tag="o")
                    nc.scalar.activation(
                        o[:], psT[:], mybir.ActivationFunctionType.Exp,
                        scale=1.0 / A_SCALE,
                    )
                    nc.sync.dma_start(
                        out[:].rearrange("(g p) i -> p g i", g=4), o[:]
                    )
    nc.finalize()
    return nc


_cache = {}


def _build_runner():
    """Compile once; return a callable(concat_inputs: dict) -> out [8, 4096]."""
    import jax
    from jax.sharding import Mesh, PartitionSpec
    from jax.experimental.shard_map import shard_map
    from concourse import bass2jax

    nc = build_bass()
    bass2jax.install_neuronx_cc_hook()

    partition_name = nc.partition_id_tensor.name if nc.partition_id_tensor else None
    in_names, out_names, out_avals, zero_out_shapes = [], [], [], []
    for alloc in nc.m.functions[0].allocations:
        if not isinstance(alloc, mybir.MemoryLocationSet):
            continue
        name = alloc.memorylocations[0].name
        if alloc.kind == "ExternalInput":
            if name != partition_name:
                in_names.append(name)
        elif alloc.kind == "ExternalOutput":
            out_names.append(name)
            out_avals.append(
                jax.core.ShapedArray(tuple(alloc.tensor_shape), mybir.dt.np(alloc.dtype))
            )
            zero_out_shapes.append((tuple(alloc.tensor_shape), mybir.dt.np(alloc.dtype)))
    n_params = len(in_names)
    all_in_names = list(in_names) + out_names
    if partition_name is not None:
        all_in_names.append(partition_name)

    def _body(*args):
        operands = list(args)
        if partition_name is not None:
            operands.append(bass2jax.partition_id_tensor())
        outs = bass2jax._bass_exec_p.bind(
            *operands,
            out_avals=tuple(out_avals),
            in_names=tuple(all_in_names),
            out_names=tuple(out_names),
            lowering_input_output_aliases=(),
            sim_require_finite=True,
            sim_require_nnan=True,
            nc=nc,
        )
        return tuple(outs)

    devices = jax.devices()[:NCORES]
    mesh = Mesh(np.asarray(devices), ("core",))
    n_outs = len(out_names)
    sharded = jax.jit(
        shard_map(
            _body,
            mesh=mesh,
            in_specs=(PartitionSpec("core"),) * (n_params + n_outs),
            out_specs=(PartitionSpec("core"),) * n_outs,
            check_rep=False,
        ),
        donate_argnums=tuple(range(n_params, n_params + n_outs)),
        keep_unused=True,
    )

    def runner(concat_inputs):
        concat_in = [concat_inputs[name] for name in in_names]
        concat_zeros = [
            np.zeros((NCORES * s[0], *s[1:]), dt) for s, dt in zero_out_shapes
        ]
        out_arrs = sharded(*concat_in, *concat_zeros)
        # single output "out": [NCORES*512, 8] = q4 transposed, node-major
        o = np.asarray(out_arrs[out_names.index("out")]).astype(np.float32)
        q4 = o.reshape(N, B).T
        return 1.0 - q4

    return runner


def _prep_inputs(preds, prob_matrix, seed_idx):
    """Host-side: build the concatenated (axis0-sharded) input arrays."""
    P = np.asarray(prob_matrix, np.float32)
    preds = np.asarray(preds, np.float32)
    seed_idx = np.asarray(seed_idx)

    # single series matrix, fp8, chunk layout A1[p, c, :] = A[128c + p, :]
    A = (P + 0.5 * P * P) * A_SCALE
    A8 = A.astype(FP8)
    A1 = np.ascontiguousarray(A8.reshape(NCHUNK, 128, N).transpose(1, 0, 2))
    A1_cat = np.ascontiguousarray(
        A1.reshape(128, NCHUNK, NCORES, SHARD).transpose(2, 0, 1, 3)
    ).reshape(NCORES * 128, NCHUNK, SHARD)

    # q0 directly in T layout: T[p, c, i] = q0[i, 128c + p]
    q0 = (1.0 - preds).astype(FP8)  # [B, N]
    q0T = np.ascontiguousarray(q0.T.reshape(NCHUNK, 128, B).transpose(1, 0, 2))
    q0_cat = np.tile(q0T, (NCORES, 1, 1))

    # bias matmul: stationary row 0 carries -lambda*C (C from the dequantized
    # A so the fp8 error cancels when q ~= const); rows 1.. the seed clamps.
    C = A8.astype(np.float32).sum(axis=0)  # = lambda * colsum
    sext = np.zeros((NEXTRA, B), np.float32)
    sext[0, :] = 1.0
    bmv_full = np.zeros((NSEEDS, N), np.float32)
    for kk in range(NSEEDS):
        sext[1 + kk, seed_idx[kk, 0]] = 1.0
        bmv_full[kk, seed_idx[kk, 1]] = -A_SCALE * BIG
    sext_cat = np.tile(sext.astype(BF16), (NCORES, 1))
    bmv = np.concatenate([-C[None, :], bmv_full], axis=0)
    bmv_cat = np.ascontiguousarray(
        bmv.reshape(NEXTRA, NCORES, SHARD).transpose(1, 0, 2)
    ).reshape(NCORES * NEXTRA, SHARD).astype(BF16)

    return {"A1": A1_cat, "q0": q0_cat, "sext": sext_cat, "bmv": bmv_cat}


def run(preds, prob_matrix, seed_idx):
    if "runner" not in _cache:
        _cache["runner"] = _build_runner()
    return _cache["runner"](_prep_inputs(preds, prob_matrix, seed_idx))


def run_prepped(concat_inputs):
    if "runner" not in _cache:
        _cache["runner"] = _build_runner()
    return _cache["runner"](concat_inputs)


def kernel(preds, prob_matrix, seed_idx):
    return run(preds, prob_matrix, seed_idx)


# revision 17
# speedup vs baseline: 2.6032x; 1.3132x over previous
# DiffusionPropagate Trainium2 Bass kernel (v8).
#
# Math: new_pred[i,a] = 1 - prod_b(1 - P[b,a]*pred[i,b]), seeds clamped to 1,
# iterated NITER times.  In the complement domain q = 1 - pred, with the
# 2-term log series log(1-x) = -(x + x^2/2) and the post-saturation fact
# q in {~0, ~1} (so q^2 ~ q), one iteration collapses to a SINGLE matmul:
#   q_new = exp(q @ A/lambda - C - BIG*seed),  A = lambda*(P + P^2/2)
# (P^2 elementwise).  C = colsum(dequant(A))/lambda is host-derived from the
# *quantized* A so the quantization error cancels when q ~= const.  The seed
# clamp is folded into the same PSUM accumulation as one extra matmul whose
# stationary rows are [-lambda*C | -lambda*BIG*node one-hots] and moving
# rows are [ones | per-seed batch one-hots]: exp(s - BIG) == 0 exactly.
#
# Distribution (8 cores): FULL REPLICATION.  The fp8 A matrix (16MB =
# 128KB/partition) fits in SBUF, so every core computes all 4096 outputs
# each iteration and no cross-core exchange is needed at all -- the
# iteration loop is [matmul phase -> exp] with zero collectives and zero
# DMAs; exp writes the fp8 moving tiles of the next iteration in place.
# A is the stationary operand ([128 b x 128 a] tiles via free Ldweights);
# the moving operand is the tiny q chunk [128 b, 8 batch], and fp8
# DoubleRow packs two b-chunks per matmul (contraction 256).  The 16MB
# A load (~47us at the 360GB/s DMA roofline) paces iteration 1; the
# remaining three iterations take ~3us each.  The last iteration ships
# raw lambda*W (f32) and the host applies exp / 1-q (like the reference
# D-vector, a pure output transform).
import numpy as np
import ml_dtypes

import concourse.mybir as mybir
import concourse.tile as tile
from concourse import bacc

NCORES = 8
B = 8
N = 4096
NITER = 4
NSEEDS = 80
NCHUNK = N // 128            # 32 contraction chunks
NEXTRA = NSEEDS + 1          # bias-matmul contraction rows
NG = 16                      # A-matrix DMA groups (pipeline with iter-1 mm)

BF16 = ml_dtypes.bfloat16
FP8 = ml_dtypes.float8_e4m3
A_SCALE = 1024.0             # keeps fp8e4m3 entries of A in the normal range
BIG = 1024.0 * 1024.0        # -lambda*BIG/lambda = -1024 in the exponent


def build_bass():
    nc = bacc.Bacc(num_devices=NCORES)
    bf = mybir.dt.bfloat16
    f32 = mybir.dt.float32
    f8 = mybir.dt.float8e4

    A_in = nc.dram_tensor("A1", [128, NCHUNK, N], f8, kind="ExternalInput")
    q_in = nc.dram_tensor("q0", [128, NCHUNK, B], f8, kind="ExternalInput")
    sx_in = nc.dram_tensor("sext", [NEXTRA, B], bf, kind="ExternalInput")
    bm_in = nc.dram_tensor("bmv", [NEXTRA, N], bf, kind="ExternalInput")
    out = nc.dram_tensor("out", [N, B], f32, kind="ExternalOutput")

    gsz = NCHUNK // NG
    with tile.TileContext(nc) as tc:
        with (
            tc.tile_pool(name="weights", bufs=1) as wpool,
            tc.tile_pool(name="work", bufs=2) as work,
            tc.tile_pool(name="psum", bufs=2, space="PSUM") as psum_pool,
        ):
            sext = wpool.tile([NEXTRA, B], bf, tag="sext")
            nc.scalar.dma_start(sext[:], sx_in[:])
            bmv = wpool.tile([NEXTRA, N], bf, tag="bmv")
            nc.scalar.dma_start(bmv[:], bm_in[:])
            A1 = wpool.tile([128, NCHUNK, N], f8, tag="A1")
            for g in range(NG):
                sl = slice(g * gsz, (g + 1) * gsz)
                eng = nc.gpsimd if g % 2 == 0 else nc.scalar
                eng.dma_start(A1[:, sl, :], A_in[:, sl, :])
            # pre-warm the ACT Exp table off the critical path
            warm = work.tile([1, B], f32, tag="warm", bufs=1)
            nc.scalar.activation(
                warm[:], sext[0:1, :], mybir.ActivationFunctionType.Exp,
                scale=1.0 / A_SCALE,
            )

            # T[p, c, i] = q[b, i] for b = 128c + p (natural chunks)
            T = work.tile([128, NCHUNK, B], f8, tag="T")
            nc.sync.dma_start(T[:], q_in[:])

            for it in range(NITER):
                # psT[a mod 128, a >> 7, batch]; fp8 DoubleRow packs chunk
                # pair (2j, 2j+1) into one matmul (contraction 256)
                psT = psum_pool.tile([128, NCHUNK, B], f32, tag="S")
                for j in range(NCHUNK // 2):
                    mv = T[:, 2 * j : 2 * j + 2, :]
                    for g in range(NCHUNK):
                        nc.tensor.matmul(
                            psT[:, g, :],
                            A1[:, 2 * j : 2 * j + 2, 128 * g : 128 * g + 128],
                            mv,
                            perf_mode=mybir.MatmulPerfMode.DoubleRow,
                            start=(j == 0),
                            stop=False,
                        )
                for g in range(NCHUNK):
                    nc.tensor.matmul(
                        psT[:, g, :], bmv[:, 128 * g : 128 * g + 128], sext[:],
                        start=False, stop=True,
                    )

                if it < NITER - 1:
                    # exp output in fp8 IS the next iteration's moving tile
                    T = work.tile([128, NCHUNK, B], f8, tag="T")
                    nc.scalar.activation(
                        T[:], psT[:], mybir.ActivationFunctionType.Exp,
                        scale=1.0 / A_SCALE,
                    )
                else:
                    o = work.tile([128, NCHUNK, B], f32, tag="o")
                    nc.scalar.activation(
                        o[:], psT[:], mybir.ActivationFunctionType.Exp,
                        scale=1.0 / A_SCALE,
                    )
                    nc.sync.dma_start(
                        out[:].rearrange("(g p) i -> p g i", p=128), o[:]
                    )
    nc.finalize()
    return nc


_cache = {}


def _build_runner():
    """Compile once; return a callable(concat_inputs: dict) -> out [8, 4096]."""
    import jax
    from jax.sharding import Mesh, PartitionSpec
    from jax.experimental.shard_map import shard_map
    from concourse import bass2jax

    nc = build_bass()
    bass2jax.install_neuronx_cc_hook()

    partition_name = nc.partition_id_tensor.name if nc.partition_id_tensor else None
    in_names, out_names, out_avals, zero_out_shapes = [], [], [], []
    for alloc in nc.m.functions[0].allocations:
        if not isinstance(alloc, mybir.MemoryLocationSet):
            continue
        name = alloc.memorylocations[0].name
        if alloc.kind == "ExternalInput":
            if name != partition_name:
                in_names.append(name)
        elif alloc.kind == "ExternalOutput":
            out_names.append(name)
            out_avals.append(
                jax.core.ShapedArray(tuple(alloc.tensor_shape), mybir.dt.np(alloc.dtype))
            )
            zero_out_shapes.append((tuple(alloc.tensor_shape), mybir.dt.np(alloc.dtype)))
    n_params = len(in_names)
    all_in_names = list(in_names) + out_names
    if partition_name is not None:
        all_in_names.append(partition_name)

    def _body(*args):
        operands = list(args)
        if partition_name is not None:
            operands.append(bass2jax.partition_id_tensor())
        outs = bass2jax._bass_exec_p.bind(
            *operands,
            out_avals=tuple(out_avals),
            in_names=tuple(all_in_names),
            out_names=tuple(out_names),
            lowering_input_output_aliases=(),
            sim_require_finite=True,
            sim_require_nnan=True,
            nc=nc,
        )
        return tuple(outs)

    devices = jax.devices()[:NCORES]
    mesh = Mesh(np.asarray(devices), ("core",))
    n_outs = len(out_names)
    sharded = jax.jit(
        shard_map(
            _body,
            mesh=mesh,
            in_specs=(PartitionSpec("core"),) * (n_params + n_outs),
            out_specs=(PartitionSpec("core"),) * n_outs,
            check_rep=False,
        ),
        donate_argnums=tuple(range(n_params, n_params + n_outs)),
        keep_unused=True,
    )

    def runner(concat_inputs):
        concat_in = [concat_inputs[name] for name in in_names]
        concat_zeros = [
            np.zeros((NCORES * s[0], *s[1:]), dt) for s, dt in zero_out_shapes
        ]
        out_arrs = sharded(*concat_in, *concat_zeros)
        # "out": [NCORES*4096, 8] of lambda*W (replicated); take core 0,
        # apply exp and the 1-q output transform on host
        q4 = np.asarray(out_arrs[out_names.index("out")])[:N]
        return (1.0 - q4.T).astype(np.float32)

    return runner


def _prep_inputs(preds, prob_matrix, seed_idx):
    """Host-side: build the concatenated (axis0-sharded) input arrays."""
    P = np.asarray(prob_matrix, np.float32)
    preds = np.asarray(preds, np.float32)
    seed_idx = np.asarray(seed_idx)

    # single series matrix, fp8, chunk layout A1[p, c, :] = A[128c + p, :]
    A = (P + 0.5 * P * P) * A_SCALE
    A8 = A.astype(FP8)
    A1 = np.ascontiguousarray(A8.reshape(NCHUNK, 128, N).transpose(1, 0, 2))
    A1_cat = np.tile(A1, (NCORES, 1, 1))

    # q0 directly in T layout: T[p, c, i] = q0[i, 128c + p]
    q0 = (1.0 - preds).astype(FP8)  # [B, N]
    q0T = np.ascontiguousarray(q0.T.reshape(NCHUNK, 128, B).transpose(1, 0, 2))
    q0_cat = np.tile(q0T, (NCORES, 1, 1))

    # bias matmul: stationary row 0 carries -lambda*C (C from the dequantized
    # A so the fp8 error cancels when q ~= const); rows 1.. the seed clamps.
    C = A8.astype(np.float32).sum(axis=0)  # = lambda * colsum
    sext = np.zeros((NEXTRA, B), np.float32)
    sext[0, :] = 1.0
    bmv = np.zeros((NEXTRA, N), np.float32)
    bmv[0, :] = -C
    for k in range(NSEEDS):
        sext[1 + k, seed_idx[k, 0]] = 1.0
        bmv[1 + k, seed_idx[k, 1]] = -A_SCALE * BIG
    sext_cat = np.tile(sext.astype(BF16), (NCORES, 1))
    bmv_cat = np.tile(bmv.astype(BF16), (NCORES, 1))

    return {"A1": A1_cat, "q0": q0_cat, "sext": sext_cat, "bmv": bmv_cat}


def run(preds, prob_matrix, seed_idx):
    if "runner" not in _cache:
        _cache["runner"] = _build_runner()
    return _cache["runner"](_prep_inputs(preds, prob_matrix, seed_idx))


def run_prepped(concat_inputs):
    if "runner" not in _cache:
        _cache["runner"] = _build_runner()
    return _cache["runner"](concat_inputs)


def kernel(preds, prob_matrix, seed_idx):
    return run(preds, prob_matrix, seed_idx)


# revision 18
# speedup vs baseline: 2.6748x; 1.0275x over previous
# DiffusionPropagate Trainium2 Bass kernel (v8).
#
# Math: new_pred[i,a] = 1 - prod_b(1 - P[b,a]*pred[i,b]), seeds clamped to 1,
# iterated NITER times.  In the complement domain q = 1 - pred, with the
# 2-term log series log(1-x) = -(x + x^2/2) and the post-saturation fact
# q in {~0, ~1} (so q^2 ~ q), one iteration collapses to a SINGLE matmul:
#   q_new = exp(q @ A/lambda - C - BIG*seed),  A = lambda*(P + P^2/2)
# (P^2 elementwise).  C = colsum(dequant(A))/lambda is host-derived from the
# *quantized* A so the quantization error cancels when q ~= const.  The seed
# clamp is folded into the same PSUM accumulation as one extra matmul whose
# stationary rows are [-lambda*C | -lambda*BIG*node one-hots] and moving
# rows are [ones | per-seed batch one-hots]: exp(s - BIG) == 0 exactly.
#
# Distribution (8 cores): FULL REPLICATION.  The fp8 A matrix (16MB =
# 128KB/partition) fits in SBUF, so every core computes all 4096 outputs
# each iteration and no cross-core exchange is needed at all -- the
# iteration loop is [matmul phase -> exp] with zero collectives and zero
# DMAs; exp writes the fp8 moving tiles of the next iteration in place.
# A is the stationary operand ([128 b x 128 a] tiles via free Ldweights);
# the moving operand is the tiny q chunk [128 b, 8 batch], and fp8
# DoubleRow packs two b-chunks per matmul (contraction 256).  The 16MB
# A load (~47us at the 360GB/s DMA roofline) paces iteration 1; the
# remaining three iterations take ~3us each.  The last iteration ships
# raw lambda*W (f32) and the host applies exp / 1-q (like the reference
# D-vector, a pure output transform).
import numpy as np
import ml_dtypes

import concourse.mybir as mybir
import concourse.tile as tile
from concourse import bacc

NCORES = 8
B = 8
N = 4096
NITER = 4
NSEEDS = 80
NCHUNK = N // 128            # 32 contraction chunks
NEXTRA = NSEEDS + 1          # bias-matmul contraction rows
NG = 16                      # A-matrix DMA groups (pipeline with iter-1 mm)

BF16 = ml_dtypes.bfloat16
FP8 = ml_dtypes.float8_e4m3
A_SCALE = 1024.0             # keeps fp8e4m3 entries of A in the normal range
BIG = 1024.0 * 1024.0        # -lambda*BIG/lambda = -1024 in the exponent


def build_bass():
    nc = bacc.Bacc(num_devices=NCORES)
    bf = mybir.dt.bfloat16
    f32 = mybir.dt.float32
    f8 = mybir.dt.float8e4

    A_in = nc.dram_tensor("A1", [128, NCHUNK, N], f8, kind="ExternalInput")
    q_in = nc.dram_tensor("q0", [128, NCHUNK, B], f8, kind="ExternalInput")
    sx_in = nc.dram_tensor("sext", [NEXTRA, B], bf, kind="ExternalInput")
    bm_in = nc.dram_tensor("bmv", [NEXTRA, N], bf, kind="ExternalInput")
    out = nc.dram_tensor("out", [128, NCHUNK * B], f32, kind="ExternalOutput")

    gsz = NCHUNK // NG
    with tile.TileContext(nc) as tc:
        with (
            tc.tile_pool(name="weights", bufs=1) as wpool,
            tc.tile_pool(name="work", bufs=2) as work,
            tc.tile_pool(name="psum", bufs=2, space="PSUM") as psum_pool,
        ):
            A1 = wpool.tile([128, NCHUNK, N], f8, tag="A1")
            for g in range(NG):
                sl = slice(g * gsz, (g + 1) * gsz)
                eng = nc.scalar if g % 2 == 0 else nc.gpsimd
                eng.dma_start(A1[:, sl, :], A_in[:, sl, :])
            sext = wpool.tile([NEXTRA, B], bf, tag="sext")
            nc.sync.dma_start(sext[:], sx_in[:])
            bmv = wpool.tile([NEXTRA, N], bf, tag="bmv")
            nc.sync.dma_start(bmv[:], bm_in[:])
            # pre-warm the ACT Exp table off the critical path
            warm = work.tile([1, B], f32, tag="warm", bufs=1)
            nc.scalar.activation(
                warm[:], sext[0:1, :], mybir.ActivationFunctionType.Exp,
                scale=1.0 / A_SCALE,
            )

            # T[p, c, i] = q[b, i] for b = 128c + p (natural chunks)
            T = work.tile([128, NCHUNK, B], f8, tag="T")
            nc.sync.dma_start(T[:], q_in[:])

            for it in range(NITER):
                # psT[a mod 128, a >> 7, batch]; fp8 DoubleRow packs chunk
                # pair (2j, 2j+1) into one matmul (contraction 256)
                psT = psum_pool.tile([128, NCHUNK, B], f32, tag="S")
                for j in range(NCHUNK // 2):
                    mv = T[:, 2 * j : 2 * j + 2, :]
                    for g in range(NCHUNK):
                        nc.tensor.matmul(
                            psT[:, g, :],
                            A1[:, 2 * j : 2 * j + 2, 128 * g : 128 * g + 128],
                            mv,
                            perf_mode=mybir.MatmulPerfMode.DoubleRow,
                            start=(j == 0),
                            stop=False,
                        )
                for g in range(NCHUNK):
                    nc.tensor.matmul(
                        psT[:, g, :], bmv[:, 128 * g : 128 * g + 128], sext[:],
                        start=False, stop=True,
                    )

                if it < NITER - 1:
                    # exp output in fp8 IS the next iteration's moving tile;
                    # two halves so the next matmul phase starts sooner
                    T = work.tile([128, NCHUNK, B], f8, tag="T")
                    h = NCHUNK // 2
                    nc.scalar.activation(
                        T[:, 0:h, :], psT[:, 0:h, :],
                        mybir.ActivationFunctionType.Exp, scale=1.0 / A_SCALE,
                    )
                    nc.scalar.activation(
                        T[:, h:NCHUNK, :], psT[:, h:NCHUNK, :],
                        mybir.ActivationFunctionType.Exp, scale=1.0 / A_SCALE,
                    )
                else:
                    o = work.tile([128, NCHUNK, B], f32, tag="o")
                    nc.scalar.activation(
                        o[:], psT[:], mybir.ActivationFunctionType.Exp,
                        scale=1.0 / A_SCALE,
                    )
                    nc.sync.dma_start(out[:], o[:])
    nc.finalize()
    return nc


_cache = {}


def _build_runner():
    """Compile once; return a callable(concat_inputs: dict) -> out [8, 4096]."""
    import jax
    from jax.sharding import Mesh, PartitionSpec
    from jax.experimental.shard_map import shard_map
    from concourse import bass2jax

    nc = build_bass()
    bass2jax.install_neuronx_cc_hook()

    partition_name = nc.partition_id_tensor.name if nc.partition_id_tensor else None
    in_names, out_names, out_avals, zero_out_shapes = [], [], [], []
    for alloc in nc.m.functions[0].allocations:
        if not isinstance(alloc, mybir.MemoryLocationSet):
            continue
        name = alloc.memorylocations[0].name
        if alloc.kind == "ExternalInput":
            if name != partition_name:
                in_names.append(name)
        elif alloc.kind == "ExternalOutput":
            out_names.append(name)
            out_avals.append(
                jax.core.ShapedArray(tuple(alloc.tensor_shape), mybir.dt.np(alloc.dtype))
            )
            zero_out_shapes.append((tuple(alloc.tensor_shape), mybir.dt.np(alloc.dtype)))
    n_params = len(in_names)
    all_in_names = list(in_names) + out_names
    if partition_name is not None:
        all_in_names.append(partition_name)

    def _body(*args):
        operands = list(args)
        if partition_name is not None:
            operands.append(bass2jax.partition_id_tensor())
        outs = bass2jax._bass_exec_p.bind(
            *operands,
            out_avals=tuple(out_avals),
            in_names=tuple(all_in_names),
            out_names=tuple(out_names),
            lowering_input_output_aliases=(),
            sim_require_finite=True,
            sim_require_nnan=True,
            nc=nc,
        )
        return tuple(outs)

    devices = jax.devices()[:NCORES]
    mesh = Mesh(np.asarray(devices), ("core",))
    n_outs = len(out_names)
    sharded = jax.jit(
        shard_map(
            _body,
            mesh=mesh,
            in_specs=(PartitionSpec("core"),) * (n_params + n_outs),
            out_specs=(PartitionSpec("core"),) * n_outs,
            check_rep=False,
        ),
        donate_argnums=tuple(range(n_params, n_params + n_outs)),
        keep_unused=True,
    )

    def runner(concat_inputs):
        concat_in = [concat_inputs[name] for name in in_names]
        concat_zeros = [
            np.zeros((NCORES * s[0], *s[1:]), dt) for s, dt in zero_out_shapes
        ]
        out_arrs = sharded(*concat_in, *concat_zeros)
        # "out": [NCORES*4096, 8] of lambda*W (replicated); take core 0,
        # apply exp and the 1-q output transform on host
        # out core 0: [128, 32*8] = q4[p, c, i] with node b = 128c + p
        o = np.asarray(out_arrs[out_names.index("out")])[:128]
        q4 = o.reshape(128, NCHUNK, B).transpose(2, 1, 0).reshape(B, N)
        # q4[i, c*128+p] ordering: (c, p) -> b = 128c + p
        return (1.0 - q4).astype(np.float32)

    return runner


def _prep_inputs(preds, prob_matrix, seed_idx):
    """Host-side: build the concatenated (axis0-sharded) input arrays."""
    P = np.asarray(prob_matrix, np.float32)
    preds = np.asarray(preds, np.float32)
    seed_idx = np.asarray(seed_idx)

    # single series matrix, fp8, chunk layout A1[p, c, :] = A[128c + p, :]
    A = (P + 0.5 * P * P) * A_SCALE
    A8 = A.astype(FP8)
    A1 = np.ascontiguousarray(A8.reshape(NCHUNK, 128, N).transpose(1, 0, 2))
    A1_cat = np.tile(A1, (NCORES, 1, 1))

    # q0 directly in T layout: T[p, c, i] = q0[i, 128c + p]
    q0 = (1.0 - preds).astype(FP8)  # [B, N]
    q0T = np.ascontiguousarray(q0.T.reshape(NCHUNK, 128, B).transpose(1, 0, 2))
    q0_cat = np.tile(q0T, (NCORES, 1, 1))

    # bias matmul: stationary row 0 carries -lambda*C (C from the dequantized
    # A so the fp8 error cancels when q ~= const); rows 1.. the seed clamps.
    C = A8.astype(np.float32).sum(axis=0)  # = lambda * colsum
    sext = np.zeros((NEXTRA, B), np.float32)
    sext[0, :] = 1.0
    bmv = np.zeros((NEXTRA, N), np.float32)
    bmv[0, :] = -C
    for k in range(NSEEDS):
        sext[1 + k, seed_idx[k, 0]] = 1.0
        bmv[1 + k, seed_idx[k, 1]] = -A_SCALE * BIG
    sext_cat = np.tile(sext.astype(BF16), (NCORES, 1))
    bmv_cat = np.tile(bmv.astype(BF16), (NCORES, 1))

    return {"A1": A1_cat, "q0": q0_cat, "sext": sext_cat, "bmv": bmv_cat}


def run(preds, prob_matrix, seed_idx):
    if "runner" not in _cache:
        _cache["runner"] = _build_runner()
    return _cache["runner"](_prep_inputs(preds, prob_matrix, seed_idx))


def run_prepped(concat_inputs):
    if "runner" not in _cache:
        _cache["runner"] = _build_runner()
    return _cache["runner"](concat_inputs)


def kernel(preds, prob_matrix, seed_idx):
    return run(preds, prob_matrix, seed_idx)
